# revision 2
# baseline (speedup 1.0000x reference)
"""Multi-head GAT layer (4 heads x 64) on 8 Trainium2 NeuronCores.

Degree-sorted identity-scatter design:
  Host sorts destination nodes by in-degree (incl. self-loop) and assigns
  each of the 128 partition slots of a destination block to exactly ONE
  destination node. Edges of a node occupy successive chunks at its slot.
  Because blocks group nodes of near-equal degree, padding waste is small.

  Consequences on device (per 128-edge chunk):
    - scatter one-hot == identity (constant stationary, no per-chunk
      one-hot generation, no PE transpose)
    - a_dst gather == direct row read (ad_store row e), pre-added with the
      pad mask into `admix` [P, C, H] once at startup
    - softmax denominator accumulates via the same identity matmul
      (w values ride in columns 256:260 of the scatter rhs)

  Per chunk: 2 matmuls h|a_s = xe^T @ [W | W@A_s] (PSUM), one pair-batched
  e = a_s + admix add (DVE), lrelu (DVE x2 or ACT), exp (ACT -> whX cols
  256:260 bf16), wh = h * w (DVE, pair-batched), identity-scatter matmul
  accumulating [wh | w] into the block accumulator. Division at block end.

  Chunks are processed in PAIRS sharing one PSUM tile [P, 2, 512] so the
  small e-chain ops amortize; scatter matmuls are emitted one pair late so
  the PE never waits on the DVE/ACT pipeline.
"""

import numpy as np
import ml_dtypes

N_NODES = 50000
IN_F = 256
H = 4
D = 64
HD = H * D
NEG_SLOPE = 0.2

P = 128
NCORES = 8
NBLK = 49
SHARD = NBLK * P          # 6272
NPAD = NCORES * SHARD     # 50176
NBLK_TOT = NPAD // P      # 392
WCOLS = 260               # W | W@A_s
LB = 16                   # chunks per edge-stream DMA batch
MASKNEG = -300.0

_BF16 = ml_dtypes.bfloat16


# ---------------------------------------------------------------------------
# Host preprocessing
# ---------------------------------------------------------------------------

def _preprocess_sorted(edge_index):
    """Degree-sort nodes, deal blocks to cores (snake), slot edges.

    Returns:
      K:         [NBLK] chunks per block slot (uniform across cores)
      sfull_all: [NCORES][C*P] int32 source node per edge slot (N_NODES=pad)
      mask_all:  [NCORES][P, C] float32 (0 real / MASKNEG pad)
      nodeid_all:[NCORES][SHARD] global node id per output row (>=N_NODES pad)
    """
    src = np.concatenate([edge_index[0], np.arange(N_NODES, dtype=np.int64)])
    dst = np.concatenate([edge_index[1], np.arange(N_NODES, dtype=np.int64)])
    deg = np.bincount(dst, minlength=NPAD).astype(np.int64)

    order = np.argsort(-deg, kind="stable")          # [NPAD] node ids
    eorder = np.argsort(dst, kind="stable")
    src_sorted = src[eorder].astype(np.int32)
    starts = np.zeros(NPAD + 1, dtype=np.int64)
    starts[1:] = np.cumsum(deg)

    # snake-deal global blocks to cores
    core_blocks = [[] for _ in range(NCORES)]
    for g in range(NBLK_TOT):
        r, i = divmod(g, NCORES)
        c = i if r % 2 == 0 else NCORES - 1 - i
        core_blocks[c].append(g)

    blk_nodes = order.reshape(NBLK_TOT, P)
    blk_K = deg[blk_nodes].max(axis=1)
    K = np.zeros(NBLK, dtype=np.int64)
    for s in range(NBLK):
        K[s] = max(1, max(blk_K[core_blocks[c][s]] for c in range(NCORES)))
    if K.sum() % 2 == 1:
        K[-1] += 1
    koff = np.concatenate([[0], np.cumsum(K)])
    C = int(koff[-1])

    sfull_all, mask_all, nodeid_all = [], [], []
    for c in range(NCORES):
        sfull = np.full(C * P, N_NODES, dtype=np.int32)
        mask = np.full((P, C), MASKNEG, dtype=np.float32)
        ids_all = np.empty(SHARD, dtype=np.int64)
        for s in range(NBLK):
            ids = blk_nodes[core_blocks[c][s]]       # [P]
            ids_all[s * P:(s + 1) * P] = ids
            degs = deg[ids]
            tot = int(degs.sum())
            if tot == 0:
                continue
            e_rep = np.repeat(np.arange(P), degs)
            off = np.zeros(P, dtype=np.int64)
            off[1:] = np.cumsum(degs)[:-1]
            j_idx = np.arange(tot) - np.repeat(off, degs)
            srcs = src_sorted[np.repeat(starts[ids], degs) + j_idx]
            pos = (koff[s] + j_idx) * P + e_rep
            sfull[pos] = srcs
            mask[e_rep, koff[s] + j_idx] = 0.0
        sfull_all.append(sfull)
        mask_all.append(mask)
        nodeid_all.append(ids_all)
    return K, sfull_all, mask_all, nodeid_all


def _edge_stream(x_b, sfull, C):
    """x_b [NPAD+1, 256] bf16 -> [P(r), C, 2(k), P(e)] bf16 where
    element (r, c, k, e) = x_b[src[c,e], 128k + r]."""
    g = x_b[sfull]                       # [C*P, 256]
    g = g.reshape(C, P, 2, P)            # [c, e, k, r]
    g = g.transpose(3, 0, 2, 1)          # [r, c, k, e]
    return np.ascontiguousarray(g)


def _host_weights(W, att_src, att_dst):
    W3 = W.reshape(IN_F, H, D)
    wa_s = np.einsum("khd,hd->kh", W3, att_src)
    wa_d = np.einsum("khd,hd->kh", W3, att_dst)
    w_ext = np.concatenate([W, wa_s], axis=1)      # [256, 260]
    return (np.ascontiguousarray(w_ext.astype(_BF16)),
            np.ascontiguousarray(wa_d.astype(_BF16)))  # [256, 4]


# ---------------------------------------------------------------------------
# Device kernel builder
# ---------------------------------------------------------------------------

def _build_nc(K, use_act_lrelu=False):
    import concourse.bass as bass
    import concourse.bacc as bacc
    import concourse.mybir as mybir
    import concourse.tile as tile
    from concourse.masks import make_identity
    from contextlib import ExitStack

    bf16 = mybir.dt.bfloat16
    f32 = mybir.dt.float32
    Alu = mybir.AluOpType
    Act = mybir.ActivationFunctionType

    K = [int(k) for k in K]
    C = sum(K)
    koff = np.concatenate([[0], np.cumsum(K)]).astype(int)
    # chunk -> (block, j)
    cblk = np.zeros(C, dtype=int)
    cj = np.zeros(C, dtype=int)
    for b in range(NBLK):
        cblk[koff[b]:koff[b + 1]] = b
        cj[koff[b]:koff[b + 1]] = np.arange(K[b])

    nc = bacc.Bacc(None, target_bir_lowering=False)
    xe_d = nc.dram_tensor("xe", [P, C, 2, P], bf16, kind="ExternalInput")
    xo_d = nc.dram_tensor("x_o", [SHARD, IN_F], bf16, kind="ExternalInput")
    wext_d = nc.dram_tensor("w_ext", [IN_F, WCOLS], bf16, kind="ExternalInput")
    wad_d = nc.dram_tensor("wad", [IN_F, H], bf16, kind="ExternalInput")
    msk_d = nc.dram_tensor("msk", [P, C], bf16, kind="ExternalInput")
    out_d = nc.dram_tensor("out", [SHARD, HD], f32, kind="ExternalOutput")

    with tile.TileContext(nc) as tc, ExitStack() as ctx:
        const = ctx.enter_context(tc.tile_pool(name="const", bufs=1))

        w_sb = const.tile([P, 2, WCOLS], bf16)
        nc.sync.dma_start(out=w_sb[:], in_=wext_d[:].rearrange("(k p) c -> p k c", p=P))
        wad_sb = const.tile([P, 2, H], bf16)
        nc.sync.dma_start(out=wad_sb[:], in_=wad_d[:].rearrange("(k p) c -> p k c", p=P))
        ident = const.tile([P, P], bf16)
        make_identity(nc, ident[:])
        msk = const.tile([P, C], bf16)
        nc.sync.dma_start(out=msk[:], in_=msk_d[:])
        xoT = const.tile([P, 2, SHARD], bf16)
        nc.sync.dma_start_transpose(xoT[:, 0, :], xo_d[:, 0:P])
        nc.sync.dma_start_transpose(xoT[:, 1, :], xo_d[:, P:2 * P])
        admix = const.tile([P, C, H], bf16)

        # ---- Phase AD: a_dst per block slot + mask -> admix -----------
        with tc.tile_pool(name="apsum", bufs=2, space="PSUM") as apsum:
            for b in range(NBLK):
                r = slice(b * P, (b + 1) * P)
                ps = apsum.tile([P, H], f32, tag="aps")
                nc.tensor.matmul(ps[:], lhsT=xoT[:, 0, r], rhs=wad_sb[:, 0, :],
                                 start=True, stop=False)
                nc.tensor.matmul(ps[:], lhsT=xoT[:, 1, r], rhs=wad_sb[:, 1, :],
                                 start=False, stop=True)
                ks = slice(int(koff[b]), int(koff[b + 1]))
                kb = K[b]
                nc.vector.tensor_tensor(
                    out=admix[:, ks, :],
                    in0=msk[:, ks][:, :, None].to_broadcast([P, kb, H]),
                    in1=ps[:, None, :].to_broadcast([P, kb, H]),
                    op=Alu.add)

        # ---- Phase E: edge pipeline -----------------------------------
        # Pair-grained (one PSUM tile [P, 2, 512] per two chunks). The
        # leaky-relu is folded into the scalar engine via
        # exp(lrelu(e)) = max(exp(e), exp(0.2 e)); the DVE only does the
        # e+admix add, the bf16 max, and the wh multiply. Scatter matmuls
        # trail by two pairs so the PE never waits on the DVE/ACT chain.
        with (
            tc.tile_pool(name="ex", bufs=3) as ex,
            tc.tile_pool(name="ewh", bufs=4) as ewh,
            tc.tile_pool(name="elr", bufs=4) as elr,
            tc.tile_pool(name="eo", bufs=2) as eo,
            tc.tile_pool(name="eph", bufs=3, space="PSUM") as eph,
            tc.tile_pool(name="eacc", bufs=2, space="PSUM") as eacc,
        ):
            from collections import deque
            acc_cur = {}
            pending = deque()   # (whX, c0)

            def emit_scatter(whX, c0):
                for jj in (0, 1):
                    c = c0 + jj
                    b, j = int(cblk[c]), int(cj[c])
                    if j == 0:
                        acc_cur[b] = eacc.tile([P, WCOLS], f32, tag="acc",
                                               name=f"acc{b}")
                    acc = acc_cur[b]
                    nc.tensor.matmul(acc[:], lhsT=ident[:], rhs=whX[:, jj, :],
                                     start=(j == 0), stop=(j == K[b] - 1))
                    if j == K[b] - 1:
                        rec = elr.tile([P, H], f32, tag="rec")
                        nc.vector.reciprocal_approx_fast(rec[:], acc[:, 256:260])
                        outt = eo.tile([P, HD], f32, tag="outt")
                        nc.vector.tensor_tensor(
                            out=outt[:].rearrange("p (h d) -> p h d", h=H),
                            in0=acc[:, 0:256].rearrange("p (h d) -> p h d", h=H),
                            in1=rec[:, :, None].to_broadcast([P, H, D]),
                            op=Alu.mult)
                        nc.sync.dma_start(out=out_d[b * P:(b + 1) * P, :],
                                          in_=outt[:])

            xe_tile = None
            npairs = C // 2
            for cp in range(npairs):
                c0 = 2 * cp
                if c0 % LB == 0:
                    bn = min(LB, C - c0)
                    xe_tile = ex.tile([P, bn, 2, P], bf16, tag="xe")
                    nc.sync.dma_start(out=xe_tile[:],
                                      in_=xe_d[:, c0:c0 + bn, :, :])

                ph2 = eph.tile([P, 2, 512], f32, tag="ph")
                for jj in (0, 1):
                    sl = xe_tile[:, (c0 + jj) % LB, :, :]
                    nc.tensor.matmul(ph2[:, jj, 0:WCOLS], lhsT=sl[:, 0, :],
                                     rhs=w_sb[:, 0, :], start=True, stop=False)
                    nc.tensor.matmul(ph2[:, jj, 0:WCOLS], lhsT=sl[:, 1, :],
                                     rhs=w_sb[:, 1, :], start=False, stop=True)

                lrA = elr.tile([P, 2, H], f32, tag="lrA")
                nc.vector.tensor_tensor(out=lrA[:], in0=ph2[:, :, 256:260],
                                        in1=admix[:, c0:c0 + 2, :], op=Alu.add)
                expA = elr.tile([P, 2, H], bf16, tag="expA")
                expB = elr.tile([P, 2, H], bf16, tag="expB")
                nc.scalar.activation(expA[:], lrA[:], Act.Exp)
                nc.scalar.activation(expB[:], lrA[:], Act.Exp, scale=NEG_SLOPE)
                whX = ewh.tile([P, 2, WCOLS], bf16, tag="wh")
                nc.vector.tensor_tensor(out=whX[:, :, 256:260], in0=expA[:],
                                        in1=expB[:], op=Alu.max)
                nc.vector.tensor_tensor(
                    out=whX[:, :, 0:256].rearrange("p j (h d) -> p j h d", h=H),
                    in0=ph2[:, :, 0:256].rearrange("p j (h d) -> p j h d", h=H),
                    in1=whX[:, :, 256:260][:, :, :, None]
                        .to_broadcast([P, 2, H, D]),
                    op=Alu.mult)

                pending.append((whX, c0))
                while len(pending) > 2:
                    emit_scatter(*pending.popleft())
            while pending:
                emit_scatter(*pending.popleft())

    nc.finalize()
    return nc


# ---------------------------------------------------------------------------
# Entry point
# ---------------------------------------------------------------------------

_cache = {}


def kernel(x, edge_index, W, att_src, att_dst, bias):
    x = np.asarray(x, dtype=np.float32)
    edge_index = np.asarray(edge_index)
    W = np.asarray(W, dtype=np.float32)
    att_src = np.asarray(att_src, dtype=np.float32)
    att_dst = np.asarray(att_dst, dtype=np.float32)
    bias = np.asarray(bias, dtype=np.float32)

    n = x.shape[0]
    assert n == N_NODES, f"kernel compiled for N={N_NODES}, got {n}"

    K, sfull_all, mask_all, nodeid_all = _preprocess_sorted(edge_index)
    C = int(np.sum(K))

    key = tuple(int(k) for k in K)
    if key not in _cache:
        _cache[key] = _build_nc(K)
    nc = _cache[key]

    x_b = np.zeros((NPAD + 1, IN_F), dtype=_BF16)
    x_b[:n] = x.astype(_BF16)
    w_ext, wad = _host_weights(W, att_src, att_dst)

    in_maps = []
    for c in range(NCORES):
        in_maps.append({
            "xe": _edge_stream(x_b, sfull_all[c], C),
            "x_o": np.ascontiguousarray(x_b[nodeid_all[c]]),
            "w_ext": w_ext,
            "wad": wad,
            "msk": np.ascontiguousarray(mask_all[c].astype(_BF16)),
        })

    from concourse.bass_utils import run_bass_kernel_spmd
    res = run_bass_kernel_spmd(nc, in_maps, core_ids=list(range(NCORES)))

    out = np.empty((n, HD), dtype=np.float32)
    for c in range(NCORES):
        ids = nodeid_all[c]
        valid = ids < n
        out[ids[valid]] = res.results[c]["out"][valid]
    return out + bias[None, :]
